# revision 7
# baseline (speedup 1.0000x reference)
"""Trainium2 Bass kernel for the retrieval-KNN module:

    h   = y @ Wy_w.T + Wy_b                      # [B,N,1024]
    dz  = dic_z @ Wz_w.T + Wz_b                  # [K,1024]
    att = softmax(h @ dz.T / sqrt(1024))         # [B,N,K]
    z   = einsum('bnk,k,ke->bne', att, prior, dic_z)

Strategy: data-parallel over B across 8 NeuronCores (T=2048 tokens/core).
All operands are pre-transposed on the HOST into the layouts the PE array
wants (contraction dim on partitions), so the device does no DMA
transposes and no f32->bf16 DRAM bounce: f32 tiles are loaded straight to
SBUF and cast in place by Scalar/Vector.

The dz GEMM is sharded over the dictionary (each core computes 512 rows
of dz^T) and shared via a chunked AllGather that overlaps with the h GEMM
and staging (DZ_MODE="ag"); DZ_MODE="local" recomputes dz fully per core.

softmax uses no max-subtraction (logits/32 are O(+-3)); the prior is
folded into the dictionary on the host (pdic = prior[:,None]*dic_z), so
z = (E @ pdic) / S with E = exp(logits/32), S = sum_k E computed by DVE
accumulation + a tiny fp32 matmul that lands S directly on token
partitions (no DRAM bounce for the normalizer).
"""

import os
import sys

import numpy as np


def _ensure_paths():
    for p in ("/opt/trn_rl_repo",):
        if p not in sys.path:
            sys.path.append(p)


_ensure_paths()

from contextlib import ExitStack  # noqa: E402

import concourse.bacc as bacc  # noqa: E402
import concourse.mybir as mybir  # noqa: E402
import concourse.tile as tile  # noqa: E402
from concourse import bass_utils  # noqa: E402
from concourse.bass import ts  # noqa: E402

F32 = mybir.dt.float32
BF16 = mybir.dt.bfloat16
AF = mybir.ActivationFunctionType

NCORES = 8
# Full problem dims (hardcoded per spec nn_Causal_v_69054484185473)
B, N, EMB = 64, 256, 1024
T = (B // NCORES) * N  # 2048 tokens per core
I, O, J, K = 1024, 1024, 2048, 4096
KL = K // NCORES  # 512 dictionary rows per core in sharded mode
SCALE = 1.0 / 32.0  # 1/sqrt(EMB)

DZ_MODE = os.environ.get("BASS_DZ_MODE", "ag")  # "ag" | "local"
NAG = int(os.environ.get("BASS_NAG", "4"))  # all-gather chunks


def build_bass(dz_mode=DZ_MODE, nag=NAG, num_devices=NCORES):
    IC, OC, JC, KC = I // 128, O // 128, J // 128, K // 128  # 8 8 16 32
    NH = 2
    TH = T // NH  # 1024 tokens per half
    TS = TH // 512  # 512-token logits slices per half
    NTT = TH // 128  # 128-token tiles per half
    NEC, EC = J // 512, 512
    AGW = KL // nag  # all-gather chunk width (k columns)

    nc = bacc.Bacc("TRN2", target_bir_lowering=False, debug=False,
                   num_devices=num_devices)
    yT = nc.dram_tensor("yT", [I, T], F32, kind="ExternalInput").ap()
    WyT = nc.dram_tensor("WyT", [I, O], F32, kind="ExternalInput").ap()
    Wy_b = nc.dram_tensor("Wy_b", [O], F32, kind="ExternalInput").ap()
    WzT = nc.dram_tensor("WzT", [J, O], F32, kind="ExternalInput").ap()
    Wz_b = nc.dram_tensor("Wz_b", [O], F32, kind="ExternalInput").ap()
    KD = KL if dz_mode == "ag" else K
    dshT = nc.dram_tensor("dshT", [J, KD], F32, kind="ExternalInput").ap()
    pdic = nc.dram_tensor("pdic", [K, J], F32, kind="ExternalInput").ap()
    z = nc.dram_tensor("z", [T, J], F32, kind="ExternalOutput").ap()

    with tile.TileContext(nc) as tc, ExitStack() as stack:
        const = stack.enter_context(tc.tile_pool(name="const", bufs=1))
        wyb = const.tile([128, OC], F32)
        nc.sync.dma_start(wyb[:], Wy_b.rearrange("(c p) -> p c", p=128))
        wzb = const.tile([128, OC], F32)
        nc.sync.dma_start(wzb[:], Wz_b.rearrange("(c p) -> p c", p=128))
        onescol = const.tile([128, 1], F32)
        nc.vector.memset(onescol[:], 1.0)
        hT = const.tile([128, OC, T], BF16)  # resident h^T for all tokens

        drp = stack.enter_context(tc.tile_pool(name="dram", bufs=1, space="DRAM"))
        if dz_mode == "ag":
            cc_ins = [drp.tile([O, AGW], BF16, name=f"ccin{p}", tag=f"ccin{p}")
                      for p in range(nag)]
            cc_outs = [drp.tile([num_devices, O, AGW], BF16, addr_space="Shared",
                                name=f"ccout{p}", tag=f"ccout{p}")
                       for p in range(nag)]
        else:
            dzT_d = drp.tile([O, K], BF16)

        mps = stack.enter_context(tc.tile_pool(name="mps", bufs=3, space="PSUM"))
        sps = stack.enter_context(tc.tile_pool(name="sps", bufs=2, space="PSUM"))

        # ---------------- prologue: stage + dz GEMM (+gather) + h GEMM
        with tc.tile_pool(name="pro", bufs=1) as pro:
            wzT_sb = pro.tile([128, JC, O], BF16)
            wyT_sb = pro.tile([128, IC, O], BF16)
            yT_sb = pro.tile([128, IC, T], BF16)

            def stage(dst, src_col_ap, eng_i, rows, width):
                # load [128, rows, width] f32 chunk, cast to bf16 into dst;
                # loads alternate over both HWDGE rings, casts over ACT/DVE
                st = pro.tile([128, 4, 1024], F32, tag="stg", name="st", bufs=3)
                stv = st[:, :rows, :width]
                (nc.sync if eng_i % 2 == 0 else nc.scalar).dma_start(
                    stv, src_col_ap)
                if eng_i % 2 == 0:
                    nc.scalar.activation(dst, stv, AF.Copy)
                else:
                    nc.vector.tensor_copy(dst, stv)

            # dictionary shard (j on partitions, k free) + Wz
            ngrp = KD // 512
            dicT_bufs = 1 if ngrp == 1 else 2
            for c in range(4):  # WzT [2048,1024] -> wzT_sb
                stage(wzT_sb[:, ts(c, 4), :],
                      WzT[ts(c, 512), :].rearrange("(c p) o -> p c o", p=128),
                      c, 4, 1024)

            def dz_group(g):
                dicT = pro.tile([128, JC, 512], BF16, tag="dicT", name="dicT",
                                bufs=dicT_bufs)
                for c in range(4):
                    st = pro.tile([128, 4, 1024], F32, tag="stg", name="st",
                                  bufs=3)
                    stv = st[:, :, :512]
                    (nc.sync if c % 2 == 0 else nc.scalar).dma_start(
                        stv, dshT[ts(c, 512), ts(g, 512)]
                        .rearrange("(c p) k -> p c k", p=128))
                    if c % 2 == 0:
                        nc.scalar.activation(dicT[:, ts(c, 4), :], stv, AF.Copy)
                    else:
                        nc.vector.tensor_copy(dicT[:, ts(c, 4), :], stv)
                for oc in range(OC):
                    ps = mps.tile([128, 512], F32, tag="mm", name="ps")
                    for jc in range(JC):
                        nc.tensor.matmul(ps[:], wzT_sb[:, jc, ts(oc, 128)],
                                         dicT[:, jc, :],
                                         start=(jc == 0), stop=(jc == JC - 1))
                    dzb = pro.tile([128, 512], BF16, tag="dzb", name="dzb",
                                   bufs=3)
                    nc.vector.tensor_scalar_add(dzb[:], ps[:],
                                                wzb[:, oc:oc + 1])
                    if dz_mode == "ag":
                        for p in range(nag):
                            nc.gpsimd.dma_start(
                                cc_ins[p][ts(oc, 128), :],
                                dzb[:, ts(p, AGW)])
                    else:
                        nc.gpsimd.dma_start(dzT_d[ts(oc, 128), ts(g, 512)],
                                            dzb[:])

            dz_group(0)
            if dz_mode == "ag":
                for p in range(nag):
                    nc.gpsimd.collective_compute(
                        "AllGather", mybir.AluOpType.bypass,
                        replica_groups=[list(range(num_devices))],
                        ins=[cc_ins[p][:, :]],
                        outs=[cc_outs[p][:, :, :]])
            else:
                for g in range(1, ngrp):
                    dz_group(g)

            # y / Wy staging + h GEMM
            for c in range(2):
                stage(wyT_sb[:, ts(c, 4), :],
                      WyT[ts(c, 512), :].rearrange("(c p) o -> p c o", p=128),
                      c, 4, 1024)
            for ic in range(IC):
                st = pro.tile([128, 4, 1024], F32, tag="stg", name="st", bufs=3)
                stv = st[:, :2, :].rearrange("p a b -> p (a b)")
                (nc.sync if ic % 2 == 0 else nc.scalar).dma_start(
                    stv, yT[ts(ic, 128), :])
                if ic % 2 == 0:
                    nc.scalar.activation(yT_sb[:, ic, :], stv, AF.Copy)
                else:
                    nc.vector.tensor_copy(yT_sb[:, ic, :], stv)
            for tcn in range(T // 512):
                for oc in range(OC):
                    ps = mps.tile([128, 512], F32, tag="mm", name="ps")
                    for ic in range(IC):
                        nc.tensor.matmul(ps[:], wyT_sb[:, ic, ts(oc, 128)],
                                         yT_sb[:, ic, ts(tcn, 512)],
                                         start=(ic == 0), stop=(ic == IC - 1))
                    nc.vector.tensor_scalar_add(hT[:, oc, ts(tcn, 512)],
                                                ps[:], wyb[:, oc:oc + 1])

        # ---------------- main: per-half logits/exp/sums then weighted sum
        epp = stack.enter_context(tc.tile_pool(name="epp", bufs=1))
        mp = stack.enter_context(tc.tile_pool(name="mp", bufs=1))
        wp = stack.enter_context(tc.tile_pool(name="wp", bufs=1))
        zp = stack.enter_context(tc.tile_pool(name="zp", bufs=1))

        if dz_mode == "ag":
            kc_order = [(r * (KL // 128) + p * (AGW // 128) + m, p)
                        for p in range(nag) for r in range(num_devices)
                        for m in range(AGW // 128)]
        else:
            kc_order = [(kc, 0) for kc in range(KC)]

        for h in range(NH):
            Ep = epp.tile([128, KC, TH], BF16, tag="Ep", name="Ep", bufs=1)
            saccs = []
            for s in range(TS):
                sacc = mp.tile([128, 512], F32, tag=f"sacc{s}", name="sacc",
                               bufs=1)
                nc.vector.memset(sacc[:], 0.0)
                saccs.append(sacc)
            def stage_pdic(ec):
                # f32 load (scalar ring) + DVE cast of pdic[:, ec] -> bf16
                pdicE = wp.tile([128, KC, EC], BF16, tag="pdicE", name="pdicE",
                                bufs=2)
                for cc in range(4):
                    pst = wp.tile([128, 8, EC], F32, tag="pst", name="pst",
                                  bufs=1)
                    nc.scalar.dma_start(
                        pst[:],
                        pdic[ts(cc, 1024), ts(ec, EC)]
                        .rearrange("(c p) e -> p c e", p=128))
                    nc.vector.tensor_copy(pdicE[:, ts(cc, 8), :], pst[:])
                return pdicE

            nextE = None
            for idx, (kc, p) in enumerate(kc_order):
                if idx == KC // 2:
                    nextE = stage_pdic(0)  # prefetch first wsum slice
                dzTk = mp.tile([128, OC, 128], BF16, tag="dzTk", name="dzTk",
                               bufs=4)
                if dz_mode == "ag":
                    r = kc // (KL // 128)
                    m0 = (kc % (KL // 128)) * 128 - p * AGW
                    nc.sync.dma_start(
                        dzTk[:],
                        cc_outs[p][r, :, m0:m0 + 128]
                        .rearrange("(c p) m -> p c m", p=128))
                else:
                    nc.sync.dma_start(
                        dzTk[:],
                        dzT_d[:, ts(kc, 128)]
                        .rearrange("(c p) m -> p c m", p=128))
                for s in range(TS):
                    ps = mps.tile([128, 512], F32, tag="mm", name="ps")
                    for oc in range(OC):
                        nc.tensor.matmul(
                            ps[:], dzTk[:, oc, :],
                            hT[:, oc, h * TH + s * 512:h * TH + (s + 1) * 512],
                            start=(oc == 0), stop=(oc == OC - 1))
                    nc.scalar.activation(Ep[:, kc, ts(s, 512)], ps[:], AF.Exp,
                                         scale=SCALE)
                    nc.vector.tensor_add(saccs[s][:], saccs[s][:],
                                         Ep[:, kc, ts(s, 512)])
            # S on token partitions: spst[t,0] = sum_lane sacc[lane, t]
            rsum = mp.tile([128, NTT], F32, tag="rsum", name="rsum", bufs=2)
            for tt in range(NTT):
                s, col = tt // 4, (tt % 4) * 128
                spst = sps.tile([128, 1], F32, tag="spst", name="spst")
                nc.tensor.matmul(spst[:], saccs[s][:, col:col + 128],
                                 onescol[:], start=True, stop=True)
                nc.vector.reciprocal(rsum[:, tt:tt + 1], spst[:])
            # weighted sum over the dictionary (pdicE staged one ec ahead)
            for ec in range(NEC):
                pdicE = nextE
                nextE = stage_pdic(ec + 1) if ec + 1 < NEC else None
                for tt in range(NTT):
                    zps = mps.tile([128, EC], F32, tag="mm", name="zps")
                    for kc in range(KC):
                        nc.tensor.matmul(zps[:], Ep[:, kc, ts(tt, 128)],
                                         pdicE[:, kc, :],
                                         start=(kc == 0), stop=(kc == KC - 1))
                    zt = zp.tile([128, EC], F32, tag="zt", name="zt", bufs=3)
                    nc.vector.tensor_scalar_mul(zt[:], zps[:],
                                                rsum[:, tt:tt + 1])
                    row0 = h * TH + tt * 128
                    (nc.gpsimd if tt % 2 == 0 else nc.scalar).dma_start(
                        z[row0:row0 + 128, ts(ec, EC)], zt[:])

    nc.compile()
    return nc


_NC_CACHE = {}


def _get_nc():
    key = (DZ_MODE, NAG)
    if key not in _NC_CACHE:
        _NC_CACHE[key] = build_bass(dz_mode=DZ_MODE, nag=NAG)
    return _NC_CACHE[key]


def make_in_maps(y, Wy_w, Wy_b, Wz_w, Wz_b, dic_z, prior):
    Bs = B // NCORES
    y = np.asarray(y, np.float32)
    dic_z = np.asarray(dic_z, np.float32)
    prior = np.asarray(prior, np.float32)
    shared = {
        "WyT": np.ascontiguousarray(np.asarray(Wy_w, np.float32).T),
        "Wy_b": np.ascontiguousarray(np.asarray(Wy_b, np.float32)),
        "WzT": np.ascontiguousarray(np.asarray(Wz_w, np.float32).T),
        "Wz_b": np.ascontiguousarray(np.asarray(Wz_b, np.float32)),
        "pdic": np.ascontiguousarray(prior[:, None] * dic_z),
    }
    if DZ_MODE == "ag":
        dsh = [np.ascontiguousarray(dic_z[i * KL:(i + 1) * KL].T)
               for i in range(NCORES)]
    else:
        full = np.ascontiguousarray(dic_z.T)
        dsh = [full] * NCORES
    return [{**shared,
             "dshT": dsh[i],
             "yT": np.ascontiguousarray(
                 y[i * Bs:(i + 1) * Bs].reshape(Bs * N, EMB).T)}
            for i in range(NCORES)]


def run_spmd(in_maps, **kw):
    nc = _get_nc()
    res = bass_utils.run_bass_kernel_spmd(nc, in_maps,
                                          core_ids=list(range(NCORES)), **kw)
    Bs = B // NCORES
    z = np.concatenate(
        [res.results[i]["z"].reshape(Bs, N, J) for i in range(NCORES)],
        axis=0)
    return z.astype(np.float32), res


def kernel(y, Wy_w, Wy_b, Wz_w, Wz_b, dic_z, prior):
    """Full-input / full-output entry point (shards over B internally)."""
    z, _ = run_spmd(make_in_maps(y, Wy_w, Wy_b, Wz_w, Wz_b, dic_z, prior))
    return z


# revision 9
# speedup vs baseline: 1.0183x; 1.0183x over previous
"""Trainium2 Bass kernel for the retrieval-KNN module:

    h   = y @ Wy_w.T + Wy_b                      # [B,N,1024]
    dz  = dic_z @ Wz_w.T + Wz_b                  # [K,1024]
    att = softmax(h @ dz.T / sqrt(1024))         # [B,N,K]
    z   = einsum('bnk,k,ke->bne', att, prior, dic_z)

Strategy: data-parallel over B across 8 NeuronCores (T=2048 tokens/core).
All operands are pre-transposed on the HOST into the layouts the PE array
wants (contraction dim on partitions), so the device does no DMA
transposes and no f32->bf16 DRAM bounce: f32 tiles are loaded straight to
SBUF and cast in place by Scalar/Vector.

The dz GEMM is sharded over the dictionary (each core computes 512 rows
of dz^T) and shared via a chunked AllGather that overlaps with the h GEMM
and staging (DZ_MODE="ag"); DZ_MODE="local" recomputes dz fully per core.

softmax uses no max-subtraction (logits/32 are O(+-3)); the prior is
folded into the dictionary on the host (pdic = prior[:,None]*dic_z), so
z = (E @ pdic) / S with E = exp(logits/32), S = sum_k E computed by DVE
accumulation + a tiny fp32 matmul that lands S directly on token
partitions (no DRAM bounce for the normalizer).
"""

import os
import sys

import numpy as np


def _ensure_paths():
    for p in ("/opt/trn_rl_repo",):
        if p not in sys.path:
            sys.path.append(p)


_ensure_paths()

from contextlib import ExitStack  # noqa: E402

import concourse.bacc as bacc  # noqa: E402
import concourse.mybir as mybir  # noqa: E402
import concourse.tile as tile  # noqa: E402
from concourse import bass_utils  # noqa: E402
from concourse.bass import ts  # noqa: E402

F32 = mybir.dt.float32
BF16 = mybir.dt.bfloat16
AF = mybir.ActivationFunctionType

NCORES = 8
# Full problem dims (hardcoded per spec nn_Causal_v_69054484185473)
B, N, EMB = 64, 256, 1024
T = (B // NCORES) * N  # 2048 tokens per core
I, O, J, K = 1024, 1024, 2048, 4096
KL = K // NCORES  # 512 dictionary rows per core in sharded mode
SCALE = 1.0 / 32.0  # 1/sqrt(EMB)

DZ_MODE = os.environ.get("BASS_DZ_MODE", "ag")  # "ag" | "local"
NAG = int(os.environ.get("BASS_NAG", "4"))  # all-gather chunks


def build_bass(dz_mode=DZ_MODE, nag=NAG, num_devices=NCORES):
    IC, OC, JC, KC = I // 128, O // 128, J // 128, K // 128  # 8 8 16 32
    NH = 2
    TH = T // NH  # 1024 tokens per half
    TS = TH // 512  # 512-token logits slices per half
    NTT = TH // 128  # 128-token tiles per half
    NEC, EC = J // 512, 512
    AGW = KL // nag  # all-gather chunk width (k columns)

    nc = bacc.Bacc("TRN2", target_bir_lowering=False, debug=False,
                   num_devices=num_devices)
    yT = nc.dram_tensor("yT", [I, T], F32, kind="ExternalInput").ap()
    WyT = nc.dram_tensor("WyT", [I, O], F32, kind="ExternalInput").ap()
    Wy_b = nc.dram_tensor("Wy_b", [O], F32, kind="ExternalInput").ap()
    WzT = nc.dram_tensor("WzT", [J, O], F32, kind="ExternalInput").ap()
    Wz_b = nc.dram_tensor("Wz_b", [O], F32, kind="ExternalInput").ap()
    KD = KL if dz_mode == "ag" else K
    dshT = nc.dram_tensor("dshT", [J, KD], F32, kind="ExternalInput").ap()
    pdic = nc.dram_tensor("pdic", [K, J], F32, kind="ExternalInput").ap()
    z = nc.dram_tensor("z", [T, J], F32, kind="ExternalOutput").ap()

    with tile.TileContext(nc) as tc, ExitStack() as stack:
        const = stack.enter_context(tc.tile_pool(name="const", bufs=1))
        wyb = const.tile([128, OC], F32)
        nc.sync.dma_start(wyb[:], Wy_b.rearrange("(c p) -> p c", p=128))
        wzb = const.tile([128, OC], F32)
        nc.sync.dma_start(wzb[:], Wz_b.rearrange("(c p) -> p c", p=128))
        onescol = const.tile([128, 1], F32)
        nc.vector.memset(onescol[:], 1.0)
        hT = const.tile([128, OC, T], BF16)  # resident h^T for all tokens

        drp = stack.enter_context(tc.tile_pool(name="dram", bufs=1, space="DRAM"))
        if dz_mode == "ag":
            cc_ins = [drp.tile([O, AGW], BF16, name=f"ccin{p}", tag=f"ccin{p}")
                      for p in range(nag)]
            cc_outs = [drp.tile([num_devices, O, AGW], BF16, addr_space="Shared",
                                name=f"ccout{p}", tag=f"ccout{p}")
                       for p in range(nag)]
        else:
            dzT_d = drp.tile([O, K], BF16)

        mps = stack.enter_context(tc.tile_pool(name="mps", bufs=3, space="PSUM"))
        sps = stack.enter_context(tc.tile_pool(name="sps", bufs=2, space="PSUM"))

        # ---------------- prologue: stage + dz GEMM (+gather) + h GEMM
        with tc.tile_pool(name="pro", bufs=1) as pro:
            wzT_sb = pro.tile([128, JC, O], BF16)
            wyT_sb = pro.tile([128, IC, O], BF16)
            yT_sb = pro.tile([128, IC, T], BF16)

            def stage(dst, src_col_ap, eng_i, rows, width):
                # load [128, rows, width] f32 chunk, cast to bf16 into dst;
                # loads alternate over both HWDGE rings, casts over ACT/DVE
                st = pro.tile([128, 4, 1024], F32, tag="stg", name="st", bufs=3)
                stv = st[:, :rows, :width]
                (nc.sync if eng_i % 2 == 0 else nc.scalar).dma_start(
                    stv, src_col_ap)
                if eng_i % 2 == 0:
                    nc.scalar.activation(dst, stv, AF.Copy)
                else:
                    nc.vector.tensor_copy(dst, stv)

            # dictionary shard (j on partitions, k free) + Wz
            ngrp = KD // 512
            dicT_bufs = 1 if ngrp == 1 else 2
            for c in range(4):  # WzT [2048,1024] -> wzT_sb
                stage(wzT_sb[:, ts(c, 4), :],
                      WzT[ts(c, 512), :].rearrange("(c p) o -> p c o", p=128),
                      c, 4, 1024)

            def dz_group(g):
                dicT = pro.tile([128, JC, 512], BF16, tag="dicT", name="dicT",
                                bufs=dicT_bufs)
                for c in range(4):
                    st = pro.tile([128, 4, 1024], F32, tag="stg", name="st",
                                  bufs=3)
                    stv = st[:, :, :512]
                    (nc.sync if c % 2 == 0 else nc.scalar).dma_start(
                        stv, dshT[ts(c, 512), ts(g, 512)]
                        .rearrange("(c p) k -> p c k", p=128))
                    if c % 2 == 0:
                        nc.scalar.activation(dicT[:, ts(c, 4), :], stv, AF.Copy)
                    else:
                        nc.vector.tensor_copy(dicT[:, ts(c, 4), :], stv)
                for oc in range(OC):
                    ps = mps.tile([128, 512], F32, tag="mm", name="ps")
                    for jc in range(JC):
                        nc.tensor.matmul(ps[:], wzT_sb[:, jc, ts(oc, 128)],
                                         dicT[:, jc, :],
                                         start=(jc == 0), stop=(jc == JC - 1))
                    dzb = pro.tile([128, 512], BF16, tag="dzb", name="dzb",
                                   bufs=3)
                    nc.vector.tensor_scalar_add(dzb[:], ps[:],
                                                wzb[:, oc:oc + 1])
                    if dz_mode == "ag":
                        for p in range(nag):
                            nc.gpsimd.dma_start(
                                cc_ins[p][ts(oc, 128), :],
                                dzb[:, ts(p, AGW)])
                    else:
                        nc.gpsimd.dma_start(dzT_d[ts(oc, 128), ts(g, 512)],
                                            dzb[:])

            dz_group(0)
            if dz_mode == "ag":
                for p in range(nag):
                    nc.gpsimd.collective_compute(
                        "AllGather", mybir.AluOpType.bypass,
                        replica_groups=[list(range(num_devices))],
                        ins=[cc_ins[p][:, :]],
                        outs=[cc_outs[p][:, :, :]])
            else:
                for g in range(1, ngrp):
                    dz_group(g)

            # y / Wy staging + h GEMM
            for c in range(2):
                stage(wyT_sb[:, ts(c, 4), :],
                      WyT[ts(c, 512), :].rearrange("(c p) o -> p c o", p=128),
                      c, 4, 1024)
            for ic in range(IC):
                st = pro.tile([128, 4, 1024], F32, tag="stg", name="st", bufs=3)
                stv = st[:, :2, :].rearrange("p a b -> p (a b)")
                (nc.sync if ic % 2 == 0 else nc.scalar).dma_start(
                    stv, yT[ts(ic, 128), :])
                if ic % 2 == 0:
                    nc.scalar.activation(yT_sb[:, ic, :], stv, AF.Copy)
                else:
                    nc.vector.tensor_copy(yT_sb[:, ic, :], stv)
            for tcn in range(T // 512):
                for oc in range(OC):
                    ps = mps.tile([128, 512], F32, tag="mm", name="ps")
                    for ic in range(IC):
                        nc.tensor.matmul(ps[:], wyT_sb[:, ic, ts(oc, 128)],
                                         yT_sb[:, ic, ts(tcn, 512)],
                                         start=(ic == 0), stop=(ic == IC - 1))
                    nc.vector.tensor_scalar_add(hT[:, oc, ts(tcn, 512)],
                                                ps[:], wyb[:, oc:oc + 1])

        # ---------------- main: per-half logits/exp/sums then weighted sum
        epp = stack.enter_context(tc.tile_pool(name="epp", bufs=1))
        mp = stack.enter_context(tc.tile_pool(name="mp", bufs=1))
        wp = stack.enter_context(tc.tile_pool(name="wp", bufs=1))
        zp = stack.enter_context(tc.tile_pool(name="zp", bufs=1))

        if dz_mode == "ag":
            kc_order = [(r * (KL // 128) + p * (AGW // 128) + m, p)
                        for p in range(nag) for r in range(num_devices)
                        for m in range(AGW // 128)]
        else:
            kc_order = [(kc, 0) for kc in range(KC)]

        for h in range(NH):
            Ep = epp.tile([128, KC, TH], BF16, tag="Ep", name="Ep", bufs=1)
            saccs = []
            for s in range(TS):
                sacc = mp.tile([128, 512], F32, tag=f"sacc{s}", name="sacc",
                               bufs=1)
                nc.vector.memset(sacc[:], 0.0)
                saccs.append(sacc)
            def stage_pdic(ec):
                # f32 load (scalar ring) + ACT cast of pdic[:, ec] -> bf16
                # (casts ride ScalarE so DVE stays free for the zt scales
                # that release wsum PSUM banks)
                pdicE = wp.tile([128, KC, EC], BF16, tag="pdicE", name="pdicE",
                                bufs=2)
                for cc in range(4):
                    pst = wp.tile([128, 8, EC], F32, tag="pst", name="pst",
                                  bufs=1)
                    nc.scalar.dma_start(
                        pst[:],
                        pdic[ts(cc, 1024), ts(ec, EC)]
                        .rearrange("(c p) e -> p c e", p=128))
                    nc.scalar.activation(pdicE[:, ts(cc, 8), :], pst[:],
                                         AF.Copy)
                return pdicE

            nextE = None
            for idx, (kc, p) in enumerate(kc_order):
                if idx == KC * 3 // 4:
                    nextE = stage_pdic(0)  # prefetch first wsum slice
                dzTk = mp.tile([128, OC, 128], BF16, tag="dzTk", name="dzTk",
                               bufs=4)
                if dz_mode == "ag":
                    r = kc // (KL // 128)
                    m0 = (kc % (KL // 128)) * 128 - p * AGW
                    nc.sync.dma_start(
                        dzTk[:],
                        cc_outs[p][r, :, m0:m0 + 128]
                        .rearrange("(c p) m -> p c m", p=128))
                else:
                    nc.sync.dma_start(
                        dzTk[:],
                        dzT_d[:, ts(kc, 128)]
                        .rearrange("(c p) m -> p c m", p=128))
                for s in range(TS):
                    ps = mps.tile([128, 512], F32, tag="mm", name="ps")
                    for oc in range(OC):
                        nc.tensor.matmul(
                            ps[:], dzTk[:, oc, :],
                            hT[:, oc, h * TH + s * 512:h * TH + (s + 1) * 512],
                            start=(oc == 0), stop=(oc == OC - 1))
                    nc.scalar.activation(Ep[:, kc, ts(s, 512)], ps[:], AF.Exp,
                                         scale=SCALE)
                    nc.vector.tensor_add(saccs[s][:], saccs[s][:],
                                         Ep[:, kc, ts(s, 512)])
            # S on token partitions: spst[t,0] = sum_lane sacc[lane, t]
            rsum = mp.tile([128, NTT], F32, tag="rsum", name="rsum", bufs=2)
            for tt in range(NTT):
                s, col = tt // 4, (tt % 4) * 128
                spst = sps.tile([128, 1], F32, tag="spst", name="spst")
                nc.tensor.matmul(spst[:], saccs[s][:, col:col + 128],
                                 onescol[:], start=True, stop=True)
                nc.vector.reciprocal(rsum[:, tt:tt + 1], spst[:])
            # weighted sum over the dictionary (pdicE staged one ec ahead)
            for ec in range(NEC):
                pdicE = nextE
                nextE = stage_pdic(ec + 1) if ec + 1 < NEC else None
                for tt in range(NTT):
                    zps = mps.tile([128, EC], F32, tag="mm", name="zps")
                    for kc in range(KC):
                        nc.tensor.matmul(zps[:], Ep[:, kc, ts(tt, 128)],
                                         pdicE[:, kc, :],
                                         start=(kc == 0), stop=(kc == KC - 1))
                    zt = zp.tile([128, EC], F32, tag="zt", name="zt", bufs=3)
                    nc.vector.tensor_scalar_mul(zt[:], zps[:],
                                                rsum[:, tt:tt + 1])
                    row0 = h * TH + tt * 128
                    nc.gpsimd.dma_start(z[row0:row0 + 128, ts(ec, EC)], zt[:])

    nc.compile()
    return nc


_NC_CACHE = {}


def _get_nc():
    key = (DZ_MODE, NAG)
    if key not in _NC_CACHE:
        _NC_CACHE[key] = build_bass(dz_mode=DZ_MODE, nag=NAG)
    return _NC_CACHE[key]


def make_in_maps(y, Wy_w, Wy_b, Wz_w, Wz_b, dic_z, prior):
    Bs = B // NCORES
    y = np.asarray(y, np.float32)
    dic_z = np.asarray(dic_z, np.float32)
    prior = np.asarray(prior, np.float32)
    shared = {
        "WyT": np.ascontiguousarray(np.asarray(Wy_w, np.float32).T),
        "Wy_b": np.ascontiguousarray(np.asarray(Wy_b, np.float32)),
        "WzT": np.ascontiguousarray(np.asarray(Wz_w, np.float32).T),
        "Wz_b": np.ascontiguousarray(np.asarray(Wz_b, np.float32)),
        "pdic": np.ascontiguousarray(prior[:, None] * dic_z),
    }
    if DZ_MODE == "ag":
        dsh = [np.ascontiguousarray(dic_z[i * KL:(i + 1) * KL].T)
               for i in range(NCORES)]
    else:
        full = np.ascontiguousarray(dic_z.T)
        dsh = [full] * NCORES
    return [{**shared,
             "dshT": dsh[i],
             "yT": np.ascontiguousarray(
                 y[i * Bs:(i + 1) * Bs].reshape(Bs * N, EMB).T)}
            for i in range(NCORES)]


def run_spmd(in_maps, **kw):
    nc = _get_nc()
    res = bass_utils.run_bass_kernel_spmd(nc, in_maps,
                                          core_ids=list(range(NCORES)), **kw)
    Bs = B // NCORES
    z = np.concatenate(
        [res.results[i]["z"].reshape(Bs, N, J) for i in range(NCORES)],
        axis=0)
    return z.astype(np.float32), res


def kernel(y, Wy_w, Wy_b, Wz_w, Wz_b, dic_z, prior):
    """Full-input / full-output entry point (shards over B internally)."""
    z, _ = run_spmd(make_in_maps(y, Wy_w, Wy_b, Wz_w, Wz_b, dic_z, prior))
    return z


# revision 13
# speedup vs baseline: 1.0706x; 1.0513x over previous
"""Trainium2 Bass kernel for the retrieval-KNN module:

    h   = y @ Wy_w.T + Wy_b                      # [B,N,1024]
    dz  = dic_z @ Wz_w.T + Wz_b                  # [K,1024]
    att = softmax(h @ dz.T / sqrt(1024))         # [B,N,K]
    z   = einsum('bnk,k,ke->bne', att, prior, dic_z)

Strategy: data-parallel over B across 8 NeuronCores (T=2048 tokens/core).
All operands are pre-transposed on the HOST into the layouts the PE array
wants (contraction dim on partitions), so the device does no DMA
transposes and no f32->bf16 DRAM bounce: f32 tiles are loaded straight to
SBUF and cast in place by Scalar/Vector.

dz^T is computed per core (DZ_MODE="local", default) straight into the
[o-partition, k] layout the logits matmul wants.  DZ_MODE="ag" instead
shards the dz GEMM over the dictionary (512 rows/core) and shares it via
a chunked AllGather overlapped with the h GEMM — fewer PE columns, but
the collective is intermittently unreliable under this runtime.

softmax uses no max-subtraction (logits/32 are O(+-3)); the prior is
folded into the dictionary on the host (pdic = prior[:,None]*dic_z), so
z = (E @ pdic) / S with E = exp(logits/32), S = sum_k E computed by DVE
accumulation + a tiny fp32 matmul that lands S directly on token
partitions (no DRAM bounce for the normalizer).
"""

import os
import sys

import numpy as np


def _ensure_paths():
    for p in ("/opt/trn_rl_repo",):
        if p not in sys.path:
            sys.path.append(p)


_ensure_paths()

from contextlib import ExitStack  # noqa: E402

import concourse.bacc as bacc  # noqa: E402
import concourse.mybir as mybir  # noqa: E402
import concourse.tile as tile  # noqa: E402
from concourse import bass_utils  # noqa: E402
from concourse.bass import ts  # noqa: E402

F32 = mybir.dt.float32
BF16 = mybir.dt.bfloat16
AF = mybir.ActivationFunctionType

NCORES = 8
# Full problem dims (hardcoded per spec nn_Causal_v_69054484185473)
B, N, EMB = 64, 256, 1024
T = (B // NCORES) * N  # 2048 tokens per core
I, O, J, K = 1024, 1024, 2048, 4096
KL = K // NCORES  # 512 dictionary rows per core in sharded mode
SCALE = 1.0 / 32.0  # 1/sqrt(EMB)

# "local" recomputes dz per core (deterministic, no collectives).  "ag"
# shards dz 8-way + chunked AllGather: ~5% faster when the collective
# behaves, but the gather intermittently races under this runtime
# (~1-in-5 runs returns garbage), so "local" is the default.
DZ_MODE = os.environ.get("BASS_DZ_MODE", "local")
NAG = int(os.environ.get("BASS_NAG", "4"))  # all-gather chunks


def build_bass(dz_mode=DZ_MODE, nag=NAG, num_devices=NCORES):
    IC, OC, JC, KC = I // 128, O // 128, J // 128, K // 128  # 8 8 16 32
    NH = 2
    TH = T // NH  # 1024 tokens per half
    TS = TH // 512  # 512-token logits slices per half
    NTT = TH // 128  # 128-token tiles per half
    NEC, EC = J // 512, 512
    AGW = KL // nag  # all-gather chunk width (k columns)

    nc = bacc.Bacc("TRN2", target_bir_lowering=False, debug=False,
                   num_devices=num_devices)
    yT = nc.dram_tensor("yT", [I, T], F32, kind="ExternalInput").ap()
    WyT = nc.dram_tensor("WyT", [I, O], F32, kind="ExternalInput").ap()
    Wy_b = nc.dram_tensor("Wy_b", [O], F32, kind="ExternalInput").ap()
    WzT = nc.dram_tensor("WzT", [J, O], F32, kind="ExternalInput").ap()
    Wz_b = nc.dram_tensor("Wz_b", [O], F32, kind="ExternalInput").ap()
    KD = KL if dz_mode == "ag" else K
    dshT = nc.dram_tensor("dshT", [J, KD], F32, kind="ExternalInput").ap()
    pdic = nc.dram_tensor("pdic", [K, J], F32, kind="ExternalInput").ap()
    z = nc.dram_tensor("z", [T, J], F32, kind="ExternalOutput").ap()

    with tile.TileContext(nc) as tc, ExitStack() as stack:
        const = stack.enter_context(tc.tile_pool(name="const", bufs=1))
        wyb = const.tile([128, OC], F32)
        nc.sync.dma_start(wyb[:], Wy_b.rearrange("(c p) -> p c", p=128))
        wzb = const.tile([128, OC], F32)
        nc.sync.dma_start(wzb[:], Wz_b.rearrange("(c p) -> p c", p=128))
        onescol = const.tile([128, 1], F32)
        nc.vector.memset(onescol[:], 1.0)
        hT = const.tile([128, OC, T], BF16)  # resident h^T for all tokens

        drp = stack.enter_context(tc.tile_pool(name="dram", bufs=1, space="DRAM"))
        if dz_mode == "ag":
            cc_ins = [drp.tile([O, AGW], BF16, name=f"ccin{p}", tag=f"ccin{p}")
                      for p in range(nag)]
            cc_outs = [drp.tile([num_devices, O, AGW], BF16, addr_space="Shared",
                                name=f"ccout{p}", tag=f"ccout{p}")
                       for p in range(nag)]
        else:
            dzT_d = drp.tile([O, K], BF16)

        mps = stack.enter_context(tc.tile_pool(name="mps", bufs=3, space="PSUM"))
        sps = stack.enter_context(tc.tile_pool(name="sps", bufs=2, space="PSUM"))

        # ---------------- prologue: stage + dz GEMM (+gather) + h GEMM
        with tc.tile_pool(name="pro", bufs=1) as pro:
            wzT_sb = pro.tile([128, JC, O], BF16)
            wyT_sb = pro.tile([128, IC, O], BF16)
            yT_sb = pro.tile([128, IC, T], BF16)

            def stage(dst, src_col_ap, eng_i, rows, width):
                # load [128, rows, width] f32 chunk, cast to bf16 into dst;
                # loads alternate over both HWDGE rings, casts over ACT/DVE
                st = pro.tile([128, 4, 1024], F32, tag="stg", name="st", bufs=3)
                stv = st[:, :rows, :width]
                (nc.sync if eng_i % 2 == 0 else nc.scalar).dma_start(
                    stv, src_col_ap)
                if eng_i % 2 == 0:
                    nc.scalar.activation(dst, stv, AF.Copy)
                else:
                    nc.vector.tensor_copy(dst, stv)

            # dictionary shard (j on partitions, k free) + Wz
            ngrp = KD // 512
            dicT_bufs = 1 if ngrp == 1 else 2
            for c in range(4):  # WzT [2048,1024] -> wzT_sb
                stage(wzT_sb[:, ts(c, 4), :],
                      WzT[ts(c, 512), :].rearrange("(c p) o -> p c o", p=128),
                      c, 4, 1024)

            def dz_group(g):
                dicT = pro.tile([128, JC, 512], BF16, tag="dicT", name="dicT",
                                bufs=dicT_bufs)
                for c in range(4):
                    st = pro.tile([128, 4, 1024], F32, tag="stg", name="st",
                                  bufs=3)
                    stv = st[:, :, :512]
                    (nc.sync if c % 2 == 0 else nc.scalar).dma_start(
                        stv, dshT[ts(c, 512), ts(g, 512)]
                        .rearrange("(c p) k -> p c k", p=128))
                    if c % 2 == 0:
                        nc.scalar.activation(dicT[:, ts(c, 4), :], stv, AF.Copy)
                    else:
                        nc.vector.tensor_copy(dicT[:, ts(c, 4), :], stv)
                for oc in range(OC):
                    ps = mps.tile([128, 512], F32, tag="mm", name="ps")
                    for jc in range(JC):
                        nc.tensor.matmul(ps[:], wzT_sb[:, jc, ts(oc, 128)],
                                         dicT[:, jc, :],
                                         start=(jc == 0), stop=(jc == JC - 1))
                    dzb = pro.tile([128, 512], BF16, tag="dzb", name="dzb",
                                   bufs=3)
                    nc.vector.tensor_scalar_add(dzb[:], ps[:],
                                                wzb[:, oc:oc + 1])
                    if dz_mode == "ag":
                        for p in range(nag):
                            nc.gpsimd.dma_start(
                                cc_ins[p][ts(oc, 128), :],
                                dzb[:, ts(p, AGW)])
                    else:
                        nc.gpsimd.dma_start(dzT_d[ts(oc, 128), ts(g, 512)],
                                            dzb[:])

            dz_group(0)
            if dz_mode == "ag":
                for p in range(nag):
                    nc.gpsimd.collective_compute(
                        "AllGather", mybir.AluOpType.bypass,
                        replica_groups=[list(range(num_devices))],
                        ins=[cc_ins[p][:, :]],
                        outs=[cc_outs[p][:, :, :]])
            else:
                for g in range(1, ngrp):
                    dz_group(g)

            # y / Wy staging + h GEMM
            for c in range(2):
                stage(wyT_sb[:, ts(c, 4), :],
                      WyT[ts(c, 512), :].rearrange("(c p) o -> p c o", p=128),
                      c, 4, 1024)
            for ic in range(IC):
                st = pro.tile([128, 4, 1024], F32, tag="stg", name="st", bufs=3)
                stv = st[:, :2, :].rearrange("p a b -> p (a b)")
                (nc.sync if ic % 2 == 0 else nc.scalar).dma_start(
                    stv, yT[ts(ic, 128), :])
                if ic % 2 == 0:
                    nc.scalar.activation(yT_sb[:, ic, :], stv, AF.Copy)
                else:
                    nc.vector.tensor_copy(yT_sb[:, ic, :], stv)
            for tcn in range(T // 512):
                for oc in range(OC):
                    ps = mps.tile([128, 512], F32, tag="mm", name="ps")
                    for ic in range(IC):
                        nc.tensor.matmul(ps[:], wyT_sb[:, ic, ts(oc, 128)],
                                         yT_sb[:, ic, ts(tcn, 512)],
                                         start=(ic == 0), stop=(ic == IC - 1))
                    nc.vector.tensor_scalar_add(hT[:, oc, ts(tcn, 512)],
                                                ps[:], wyb[:, oc:oc + 1])

        # ---------------- main: per-half logits/exp/sums then weighted sum
        epp = stack.enter_context(tc.tile_pool(name="epp", bufs=1))
        mp = stack.enter_context(tc.tile_pool(name="mp", bufs=1))
        wp = stack.enter_context(tc.tile_pool(name="wp", bufs=1))
        zp = stack.enter_context(tc.tile_pool(name="zp", bufs=1))

        if dz_mode == "ag":
            kc_order = [(r * (KL // 128) + p * (AGW // 128) + m, p)
                        for p in range(nag) for r in range(num_devices)
                        for m in range(AGW // 128)]
        else:
            kc_order = [(kc, 0) for kc in range(KC)]

        for h in range(NH):
            Ep = epp.tile([128, KC, TH], BF16, tag="Ep", name="Ep", bufs=1)
            saccs = []
            for s in range(TS):
                sacc = mp.tile([128, 512], F32, tag=f"sacc{s}", name="sacc",
                               bufs=1)
                nc.vector.memset(sacc[:], 0.0)
                saccs.append(sacc)
            for kc, p in kc_order:
                dzTk = mp.tile([128, OC, 128], BF16, tag="dzTk", name="dzTk",
                               bufs=4)
                if dz_mode == "ag":
                    r = kc // (KL // 128)
                    m0 = (kc % (KL // 128)) * 128 - p * AGW
                    nc.sync.dma_start(
                        dzTk[:],
                        cc_outs[p][r, :, m0:m0 + 128]
                        .rearrange("(c p) m -> p c m", p=128))
                else:
                    nc.sync.dma_start(
                        dzTk[:],
                        dzT_d[:, ts(kc, 128)]
                        .rearrange("(c p) m -> p c m", p=128))
                for s in range(TS):
                    ps = mps.tile([128, 512], F32, tag="mm", name="ps")
                    for oc in range(OC):
                        nc.tensor.matmul(
                            ps[:], dzTk[:, oc, :],
                            hT[:, oc, h * TH + s * 512:h * TH + (s + 1) * 512],
                            start=(oc == 0), stop=(oc == OC - 1))
                    nc.scalar.activation(Ep[:, kc, ts(s, 512)], ps[:], AF.Exp,
                                         scale=SCALE)
                    nc.vector.tensor_add(saccs[s][:], saccs[s][:],
                                         Ep[:, kc, ts(s, 512)])
            # S on token partitions: spst[t,0] = sum_lane sacc[lane, t]
            rsum = mp.tile([128, NTT], F32, tag="rsum", name="rsum", bufs=2)
            for tt in range(NTT):
                s, col = tt // 4, (tt % 4) * 128
                spst = sps.tile([128, 1], F32, tag="spst", name="spst")
                nc.tensor.matmul(spst[:], saccs[s][:, col:col + 128],
                                 onescol[:], start=True, stop=True)
                nc.vector.reciprocal(rsum[:, tt:tt + 1], spst[:])
            # weighted sum over the dictionary
            for ec in range(NEC):
                pdicE = wp.tile([128, KC, EC], BF16, tag="pdicE", name="pdicE",
                                bufs=2)
                for cc in range(4):
                    pst = wp.tile([128, 8, EC], F32, tag="pst", name="pst",
                                  bufs=1)
                    nc.sync.dma_start(
                        pst[:],
                        pdic[ts(cc, 1024), ts(ec, EC)]
                        .rearrange("(c p) e -> p c e", p=128))
                    nc.vector.tensor_copy(pdicE[:, ts(cc, 8), :], pst[:])
                for tt in range(NTT):
                    zps = mps.tile([128, EC], F32, tag="mm", name="zps")
                    for kc in range(KC):
                        nc.tensor.matmul(zps[:], Ep[:, kc, ts(tt, 128)],
                                         pdicE[:, kc, :],
                                         start=(kc == 0), stop=(kc == KC - 1))
                    zt = zp.tile([128, EC], F32, tag="zt", name="zt", bufs=3)
                    nc.vector.tensor_scalar_mul(zt[:], zps[:],
                                                rsum[:, tt:tt + 1])
                    row0 = h * TH + tt * 128
                    nc.gpsimd.dma_start(z[row0:row0 + 128, ts(ec, EC)], zt[:])

    nc.compile()
    return nc


_NC_CACHE = {}


def _get_nc():
    key = (DZ_MODE, NAG)
    if key not in _NC_CACHE:
        _NC_CACHE[key] = build_bass(dz_mode=DZ_MODE, nag=NAG)
    return _NC_CACHE[key]


def make_in_maps(y, Wy_w, Wy_b, Wz_w, Wz_b, dic_z, prior):
    Bs = B // NCORES
    y = np.asarray(y, np.float32)
    dic_z = np.asarray(dic_z, np.float32)
    prior = np.asarray(prior, np.float32)
    shared = {
        "WyT": np.ascontiguousarray(np.asarray(Wy_w, np.float32).T),
        "Wy_b": np.ascontiguousarray(np.asarray(Wy_b, np.float32)),
        "WzT": np.ascontiguousarray(np.asarray(Wz_w, np.float32).T),
        "Wz_b": np.ascontiguousarray(np.asarray(Wz_b, np.float32)),
        "pdic": np.ascontiguousarray(prior[:, None] * dic_z),
    }
    if DZ_MODE == "ag":
        dsh = [np.ascontiguousarray(dic_z[i * KL:(i + 1) * KL].T)
               for i in range(NCORES)]
    else:
        full = np.ascontiguousarray(dic_z.T)
        dsh = [full] * NCORES
    return [{**shared,
             "dshT": dsh[i],
             "yT": np.ascontiguousarray(
                 y[i * Bs:(i + 1) * Bs].reshape(Bs * N, EMB).T)}
            for i in range(NCORES)]


def run_spmd(in_maps, **kw):
    nc = _get_nc()
    res = bass_utils.run_bass_kernel_spmd(nc, in_maps,
                                          core_ids=list(range(NCORES)), **kw)
    Bs = B // NCORES
    z = np.concatenate(
        [res.results[i]["z"].reshape(Bs, N, J) for i in range(NCORES)],
        axis=0)
    return z.astype(np.float32), res


def kernel(y, Wy_w, Wy_b, Wz_w, Wz_b, dic_z, prior):
    """Full-input / full-output entry point (shards over B internally)."""
    z, _ = run_spmd(make_in_maps(y, Wy_w, Wy_b, Wz_w, Wz_b, dic_z, prior))
    return z


# revision 14
# speedup vs baseline: 1.0759x; 1.0050x over previous
"""Trainium2 Bass kernel for the retrieval-KNN module:

    h   = y @ Wy_w.T + Wy_b                      # [B,N,1024]
    dz  = dic_z @ Wz_w.T + Wz_b                  # [K,1024]
    att = softmax(h @ dz.T / sqrt(1024))         # [B,N,K]
    z   = einsum('bnk,k,ke->bne', att, prior, dic_z)

Strategy: data-parallel over B across 8 NeuronCores (T=2048 tokens/core).
All operands are pre-transposed on the HOST into the layouts the PE array
wants (contraction dim on partitions), so the device does no DMA
transposes and no f32->bf16 DRAM bounce: f32 tiles are loaded straight to
SBUF and cast in place by Scalar/Vector.

dz^T is computed per core (DZ_MODE="local", default) straight into the
[o-partition, k] layout the logits matmul wants.  DZ_MODE="ag" instead
shards the dz GEMM over the dictionary (512 rows/core) and shares it via
a chunked AllGather overlapped with the h GEMM — fewer PE columns, but
the collective is intermittently unreliable under this runtime.

softmax uses no max-subtraction (logits/32 are O(+-3)); the prior is
folded into the dictionary on the host (pdic = prior[:,None]*dic_z), so
z = (E @ pdic) / S with E = exp(logits/32), S = sum_k E computed by DVE
accumulation + a tiny fp32 matmul that lands S directly on token
partitions (no DRAM bounce for the normalizer).
"""

import os
import sys

import numpy as np


def _ensure_paths():
    for p in ("/opt/trn_rl_repo",):
        if p not in sys.path:
            sys.path.append(p)


_ensure_paths()

from contextlib import ExitStack  # noqa: E402

import concourse.bacc as bacc  # noqa: E402
import concourse.mybir as mybir  # noqa: E402
import concourse.tile as tile  # noqa: E402
from concourse import bass_utils  # noqa: E402
from concourse.bass import ts  # noqa: E402

F32 = mybir.dt.float32
BF16 = mybir.dt.bfloat16
AF = mybir.ActivationFunctionType

NCORES = 8
# Full problem dims (hardcoded per spec nn_Causal_v_69054484185473)
B, N, EMB = 64, 256, 1024
T = (B // NCORES) * N  # 2048 tokens per core
I, O, J, K = 1024, 1024, 2048, 4096
KL = K // NCORES  # 512 dictionary rows per core in sharded mode
SCALE = 1.0 / 32.0  # 1/sqrt(EMB)

# "local" recomputes dz per core (deterministic, no collectives).  "ag"
# shards dz 8-way + chunked AllGather: ~5% faster when the collective
# behaves, but the gather intermittently races under this runtime
# (~1-in-5 runs returns garbage), so "local" is the default.
DZ_MODE = os.environ.get("BASS_DZ_MODE", "local")
NAG = int(os.environ.get("BASS_NAG", "4"))  # all-gather chunks


def build_bass(dz_mode=DZ_MODE, nag=NAG, num_devices=NCORES):
    IC, OC, JC, KC = I // 128, O // 128, J // 128, K // 128  # 8 8 16 32
    NH = 2
    TH = T // NH  # 1024 tokens per half
    TS = TH // 512  # 512-token logits slices per half
    NTT = TH // 128  # 128-token tiles per half
    NEC, EC = J // 512, 512
    AGW = KL // nag  # all-gather chunk width (k columns)

    nc = bacc.Bacc("TRN2", target_bir_lowering=False, debug=False,
                   num_devices=num_devices)
    yT = nc.dram_tensor("yT", [I, T], F32, kind="ExternalInput").ap()
    WyT = nc.dram_tensor("WyT", [I, O], F32, kind="ExternalInput").ap()
    Wy_b = nc.dram_tensor("Wy_b", [O], F32, kind="ExternalInput").ap()
    WzT = nc.dram_tensor("WzT", [J, O], F32, kind="ExternalInput").ap()
    Wz_b = nc.dram_tensor("Wz_b", [O], F32, kind="ExternalInput").ap()
    KD = KL if dz_mode == "ag" else K
    dshT = nc.dram_tensor("dshT", [J, KD], F32, kind="ExternalInput").ap()
    pdic = nc.dram_tensor("pdic", [K, J], F32, kind="ExternalInput").ap()
    z = nc.dram_tensor("z", [T, J], F32, kind="ExternalOutput").ap()

    with tile.TileContext(nc) as tc, ExitStack() as stack:
        const = stack.enter_context(tc.tile_pool(name="const", bufs=1))
        wyb = const.tile([128, OC], F32)
        nc.sync.dma_start(wyb[:], Wy_b.rearrange("(c p) -> p c", p=128))
        wzb = const.tile([128, OC], F32)
        nc.sync.dma_start(wzb[:], Wz_b.rearrange("(c p) -> p c", p=128))
        onescol = const.tile([128, 1], F32)
        nc.vector.memset(onescol[:], 1.0)
        hT = const.tile([128, OC, T], BF16)  # resident h^T for all tokens

        drp = stack.enter_context(tc.tile_pool(name="dram", bufs=1, space="DRAM"))
        if dz_mode == "ag":
            cc_ins = [drp.tile([O, AGW], BF16, name=f"ccin{p}", tag=f"ccin{p}")
                      for p in range(nag)]
            # Local (per-core private) gather outputs: a Shared output buffer
            # is written concurrently by every rank's gather machinery and
            # intermittently raced our dzTk reads; Local is a bit more wire
            # but raceable only by the collective's own sync, which is sound.
            cc_outs = [drp.tile([num_devices, O, AGW], BF16, addr_space="Local",
                                name=f"ccout{p}", tag=f"ccout{p}")
                       for p in range(nag)]
        else:
            dzT_d = drp.tile([O, K], BF16)

        mps = stack.enter_context(tc.tile_pool(name="mps", bufs=3, space="PSUM"))
        sps = stack.enter_context(tc.tile_pool(name="sps", bufs=2, space="PSUM"))

        # ---------------- prologue: stage + dz GEMM (+gather) + h GEMM
        with tc.tile_pool(name="pro", bufs=1) as pro:
            wzT_sb = pro.tile([128, JC, O], BF16)
            wyT_sb = pro.tile([128, IC, O], BF16)
            yT_sb = pro.tile([128, IC, T], BF16)

            def stage(dst, src_col_ap, eng_i, rows, width):
                # load [128, rows, width] f32 chunk, cast to bf16 into dst;
                # loads alternate over both HWDGE rings, casts over ACT/DVE
                st = pro.tile([128, 4, 1024], F32, tag="stg", name="st", bufs=3)
                stv = st[:, :rows, :width]
                (nc.sync if eng_i % 2 == 0 else nc.scalar).dma_start(
                    stv, src_col_ap)
                if eng_i % 2 == 0:
                    nc.scalar.activation(dst, stv, AF.Copy)
                else:
                    nc.vector.tensor_copy(dst, stv)

            # dictionary shard (j on partitions, k free) + Wz
            ngrp = KD // 512
            dicT_bufs = 1 if ngrp == 1 else 2
            for c in range(4):  # WzT [2048,1024] -> wzT_sb
                stage(wzT_sb[:, ts(c, 4), :],
                      WzT[ts(c, 512), :].rearrange("(c p) o -> p c o", p=128),
                      c, 4, 1024)

            def dz_group(g):
                dicT = pro.tile([128, JC, 512], BF16, tag="dicT", name="dicT",
                                bufs=dicT_bufs)
                for c in range(4):
                    st = pro.tile([128, 4, 1024], F32, tag="stg", name="st",
                                  bufs=3)
                    stv = st[:, :, :512]
                    (nc.sync if c % 2 == 0 else nc.scalar).dma_start(
                        stv, dshT[ts(c, 512), ts(g, 512)]
                        .rearrange("(c p) k -> p c k", p=128))
                    if c % 2 == 0:
                        nc.scalar.activation(dicT[:, ts(c, 4), :], stv, AF.Copy)
                    else:
                        nc.vector.tensor_copy(dicT[:, ts(c, 4), :], stv)
                for oc in range(OC):
                    ps = mps.tile([128, 512], F32, tag="mm", name="ps")
                    for jc in range(JC):
                        nc.tensor.matmul(ps[:], wzT_sb[:, jc, ts(oc, 128)],
                                         dicT[:, jc, :],
                                         start=(jc == 0), stop=(jc == JC - 1))
                    dzb = pro.tile([128, 512], BF16, tag="dzb", name="dzb",
                                   bufs=3)
                    nc.vector.tensor_scalar_add(dzb[:], ps[:],
                                                wzb[:, oc:oc + 1])
                    if dz_mode == "ag":
                        for p in range(nag):
                            nc.gpsimd.dma_start(
                                cc_ins[p][ts(oc, 128), :],
                                dzb[:, ts(p, AGW)])
                    else:
                        nc.gpsimd.dma_start(dzT_d[ts(oc, 128), ts(g, 512)],
                                            dzb[:])

            dz_group(0)
            if dz_mode == "ag":
                for p in range(nag):
                    nc.gpsimd.collective_compute(
                        "AllGather", mybir.AluOpType.bypass,
                        replica_groups=[list(range(num_devices))],
                        ins=[cc_ins[p][:, :]],
                        outs=[cc_outs[p][:, :, :]])
            else:
                for g in range(1, ngrp):
                    dz_group(g)

            # y / Wy staging + h GEMM
            for c in range(2):
                stage(wyT_sb[:, ts(c, 4), :],
                      WyT[ts(c, 512), :].rearrange("(c p) o -> p c o", p=128),
                      c, 4, 1024)
            for ic in range(IC):
                st = pro.tile([128, 4, 1024], F32, tag="stg", name="st", bufs=3)
                stv = st[:, :2, :].rearrange("p a b -> p (a b)")
                (nc.sync if ic % 2 == 0 else nc.scalar).dma_start(
                    stv, yT[ts(ic, 128), :])
                if ic % 2 == 0:
                    nc.scalar.activation(yT_sb[:, ic, :], stv, AF.Copy)
                else:
                    nc.vector.tensor_copy(yT_sb[:, ic, :], stv)
            for tcn in range(T // 512):
                for oc in range(OC):
                    ps = mps.tile([128, 512], F32, tag="mm", name="ps")
                    for ic in range(IC):
                        nc.tensor.matmul(ps[:], wyT_sb[:, ic, ts(oc, 128)],
                                         yT_sb[:, ic, ts(tcn, 512)],
                                         start=(ic == 0), stop=(ic == IC - 1))
                    nc.vector.tensor_scalar_add(hT[:, oc, ts(tcn, 512)],
                                                ps[:], wyb[:, oc:oc + 1])

        # ---------------- main: per-half logits/exp/sums then weighted sum
        epp = stack.enter_context(tc.tile_pool(name="epp", bufs=1))
        mp = stack.enter_context(tc.tile_pool(name="mp", bufs=1))
        wp = stack.enter_context(tc.tile_pool(name="wp", bufs=1))
        zp = stack.enter_context(tc.tile_pool(name="zp", bufs=1))

        if dz_mode == "ag":
            kc_order = [(r * (KL // 128) + p * (AGW // 128) + m, p)
                        for p in range(nag) for r in range(num_devices)
                        for m in range(AGW // 128)]
        else:
            kc_order = [(kc, 0) for kc in range(KC)]

        for h in range(NH):
            Ep = epp.tile([128, KC, TH], BF16, tag="Ep", name="Ep", bufs=1)
            saccs = []
            for s in range(TS):
                sacc = mp.tile([128, 512], F32, tag=f"sacc{s}", name="sacc",
                               bufs=1)
                nc.vector.memset(sacc[:], 0.0)
                saccs.append(sacc)
            for kc, p in kc_order:
                dzTk = mp.tile([128, OC, 128], BF16, tag="dzTk", name="dzTk",
                               bufs=4)
                if dz_mode == "ag":
                    r = kc // (KL // 128)
                    m0 = (kc % (KL // 128)) * 128 - p * AGW
                    nc.sync.dma_start(
                        dzTk[:],
                        cc_outs[p][r, :, m0:m0 + 128]
                        .rearrange("(c p) m -> p c m", p=128))
                else:
                    nc.sync.dma_start(
                        dzTk[:],
                        dzT_d[:, ts(kc, 128)]
                        .rearrange("(c p) m -> p c m", p=128))
                for s in range(TS):
                    ps = mps.tile([128, 512], F32, tag="mm", name="ps")
                    for oc in range(OC):
                        nc.tensor.matmul(
                            ps[:], dzTk[:, oc, :],
                            hT[:, oc, h * TH + s * 512:h * TH + (s + 1) * 512],
                            start=(oc == 0), stop=(oc == OC - 1))
                    nc.scalar.activation(Ep[:, kc, ts(s, 512)], ps[:], AF.Exp,
                                         scale=SCALE)
                    nc.vector.tensor_add(saccs[s][:], saccs[s][:],
                                         Ep[:, kc, ts(s, 512)])
            # S on token partitions: spst[t,0] = sum_lane sacc[lane, t]
            rsum = mp.tile([128, NTT], F32, tag="rsum", name="rsum", bufs=2)
            for tt in range(NTT):
                s, col = tt // 4, (tt % 4) * 128
                spst = sps.tile([128, 1], F32, tag="spst", name="spst")
                nc.tensor.matmul(spst[:], saccs[s][:, col:col + 128],
                                 onescol[:], start=True, stop=True)
                nc.vector.reciprocal(rsum[:, tt:tt + 1], spst[:])
            # weighted sum over the dictionary
            for ec in range(NEC):
                pdicE = wp.tile([128, KC, EC], BF16, tag="pdicE", name="pdicE",
                                bufs=2)
                for cc in range(4):
                    pst = wp.tile([128, 8, EC], F32, tag="pst", name="pst",
                                  bufs=1)
                    nc.sync.dma_start(
                        pst[:],
                        pdic[ts(cc, 1024), ts(ec, EC)]
                        .rearrange("(c p) e -> p c e", p=128))
                    nc.vector.tensor_copy(pdicE[:, ts(cc, 8), :], pst[:])
                for tt in range(NTT):
                    zps = mps.tile([128, EC], F32, tag="mm", name="zps")
                    for kc in range(KC):
                        nc.tensor.matmul(zps[:], Ep[:, kc, ts(tt, 128)],
                                         pdicE[:, kc, :],
                                         start=(kc == 0), stop=(kc == KC - 1))
                    zt = zp.tile([128, EC], F32, tag="zt", name="zt", bufs=3)
                    nc.vector.tensor_scalar_mul(zt[:], zps[:],
                                                rsum[:, tt:tt + 1])
                    row0 = h * TH + tt * 128
                    nc.gpsimd.dma_start(z[row0:row0 + 128, ts(ec, EC)], zt[:])

    nc.compile()
    return nc


_NC_CACHE = {}


def _get_nc():
    key = (DZ_MODE, NAG)
    if key not in _NC_CACHE:
        _NC_CACHE[key] = build_bass(dz_mode=DZ_MODE, nag=NAG)
    return _NC_CACHE[key]


def make_in_maps(y, Wy_w, Wy_b, Wz_w, Wz_b, dic_z, prior):
    Bs = B // NCORES
    y = np.asarray(y, np.float32)
    dic_z = np.asarray(dic_z, np.float32)
    prior = np.asarray(prior, np.float32)
    shared = {
        "WyT": np.ascontiguousarray(np.asarray(Wy_w, np.float32).T),
        "Wy_b": np.ascontiguousarray(np.asarray(Wy_b, np.float32)),
        "WzT": np.ascontiguousarray(np.asarray(Wz_w, np.float32).T),
        "Wz_b": np.ascontiguousarray(np.asarray(Wz_b, np.float32)),
        "pdic": np.ascontiguousarray(prior[:, None] * dic_z),
    }
    if DZ_MODE == "ag":
        dsh = [np.ascontiguousarray(dic_z[i * KL:(i + 1) * KL].T)
               for i in range(NCORES)]
    else:
        full = np.ascontiguousarray(dic_z.T)
        dsh = [full] * NCORES
    return [{**shared,
             "dshT": dsh[i],
             "yT": np.ascontiguousarray(
                 y[i * Bs:(i + 1) * Bs].reshape(Bs * N, EMB).T)}
            for i in range(NCORES)]


def run_spmd(in_maps, **kw):
    nc = _get_nc()
    res = bass_utils.run_bass_kernel_spmd(nc, in_maps,
                                          core_ids=list(range(NCORES)), **kw)
    Bs = B // NCORES
    z = np.concatenate(
        [res.results[i]["z"].reshape(Bs, N, J) for i in range(NCORES)],
        axis=0)
    return z.astype(np.float32), res


def kernel(y, Wy_w, Wy_b, Wz_w, Wz_b, dic_z, prior):
    """Full-input / full-output entry point (shards over B internally)."""
    z, _ = run_spmd(make_in_maps(y, Wy_w, Wy_b, Wz_w, Wz_b, dic_z, prior))
    return z
